# revision 1
# baseline (speedup 1.0000x reference)
"""Expert-parallel MoE SwiGLU FFN kernel for 8 Trainium2 NeuronCores.

Problem: T=4096 tokens, DIM=1024, E=8 experts, INTER=1408, top-2 routing.
Reference computes all experts densely then gathers; we instead route on the
host (sort token-slots by expert), assign one expert per core, and each core
runs a SwiGLU FFN over only its routed tokens (padded to a common capacity so
all 8 cores execute the same SPMD program).

Device layout (per core, everything "transposed" with tokens on the free dim):
  xt  [8,128,C]  bf16   x_gathered.T tiled over DIM      (k-tile, partition, token)
  w1t [8,128,1408] bf16 w1[e].T tiled over DIM
  w3t [8,128,1408] bf16
  w2t [11,128,1024] bf16 w2[e].T tiled over INTER
  yt  [8,128,C]  f32    y.T tiled over DIM (output)

Compute per core:
  h1.T = w1 @ x.T   (accumulate over 8 DIM k-tiles)     -> PSUM [128, n]
  h3.T = w3 @ x.T
  g.T  = silu(h1.T) * h3.T                              -> SBUF bf16
  y.T  = w2 @ g.T   (accumulate over 11 INTER m-tiles)  -> PSUM -> SBUF f32 -> HBM
"""

import numpy as np
import ml_dtypes

T, DIM, E, INTER, TOPK = 4096, 1024, 8, 1408, 2
NCORES = 8
P = 128
KT = DIM // P    # 8 k-tiles over DIM
MT = INTER // P  # 11 m-tiles over INTER

TRACE = False  # test.py sets this to capture an NTFF profile
LAST_RESULTS = None  # BassKernelResults of the last run (for test.py)

_NC_CACHE = {}


def _chunks_for(C):
    out = [512] * (C // 512)
    if C % 512:
        out.append(C % 512)
    return out


def _build_nc(C):
    import concourse.mybir as mybir
    import concourse.tile as tile
    from concourse import bacc

    dt = mybir.dt
    AF = mybir.ActivationFunctionType
    chunks = _chunks_for(C)

    nc = bacc.Bacc("TRN2", target_bir_lowering=False, debug=False)
    xt = nc.dram_tensor("xt", [KT, P, C], dt.bfloat16, kind="ExternalInput")
    w1t = nc.dram_tensor("w1t", [KT, P, INTER], dt.bfloat16, kind="ExternalInput")
    w3t = nc.dram_tensor("w3t", [KT, P, INTER], dt.bfloat16, kind="ExternalInput")
    w2t = nc.dram_tensor("w2t", [MT, P, DIM], dt.bfloat16, kind="ExternalInput")
    yt = nc.dram_tensor("yt", [KT, P, C], dt.float32, kind="ExternalOutput")

    with tile.TileContext(nc) as tc:
        with (
            tc.tile_pool(name="persist", bufs=1) as wpool,
            tc.tile_pool(name="gbuf", bufs=2) as gpool,
            tc.tile_pool(name="ybuf", bufs=3) as ypool,
            tc.tile_pool(name="silbuf", bufs=3) as spool,
            tc.tile_pool(name="psA", bufs=2, space="PSUM") as psA,
            tc.tile_pool(name="psB", bufs=2, space="PSUM") as psB,
        ):
            xs = wpool.tile([P, KT, C], dt.bfloat16)
            w1s = wpool.tile([P, KT, INTER], dt.bfloat16)
            w3s = wpool.tile([P, KT, INTER], dt.bfloat16)
            w2s = wpool.tile([P, MT, DIM], dt.bfloat16)
            for k in range(KT):
                nc.sync.dma_start(xs[:, k, :], xt[k])
                nc.sync.dma_start(w1s[:, k, :], w1t[k])
                nc.sync.dma_start(w3s[:, k, :], w3t[k])
            for m in range(MT):
                nc.sync.dma_start(w2s[:, m, :], w2t[m])

            c0 = 0
            for n in chunks:
                gs = gpool.tile([P, MT, n], dt.bfloat16, name="gs")
                for m in range(MT):
                    p1 = psA.tile([P, n], dt.float32, name="p1")
                    p3 = psA.tile([P, n], dt.float32, name="p3")
                    for k in range(KT):
                        nc.tensor.matmul(
                            p1[:],
                            w1s[:, k, m * P:(m + 1) * P],
                            xs[:, k, c0:c0 + n],
                            start=(k == 0),
                            stop=(k == KT - 1),
                        )
                    for k in range(KT):
                        nc.tensor.matmul(
                            p3[:],
                            w3s[:, k, m * P:(m + 1) * P],
                            xs[:, k, c0:c0 + n],
                            start=(k == 0),
                            stop=(k == KT - 1),
                        )
                    sil = spool.tile([P, n], dt.bfloat16, name="sil")
                    nc.scalar.activation(sil[:], p1[:], AF.Silu)
                    nc.vector.tensor_mul(gs[:, m, :], sil[:], p3[:])
                for i in range(KT):
                    py = psB.tile([P, n], dt.float32, name="py")
                    for m in range(MT):
                        nc.tensor.matmul(
                            py[:],
                            w2s[:, m, i * P:(i + 1) * P],
                            gs[:, m, :],
                            start=(m == 0),
                            stop=(m == MT - 1),
                        )
                    ys = ypool.tile([P, n], dt.float32, name="ys")
                    nc.scalar.copy(ys[:], py[:])
                    nc.sync.dma_start(yt[i, :, c0:c0 + n], ys[:])
                c0 += n

    nc.compile()
    return nc


def _get_nc(C):
    if C not in _NC_CACHE:
        _NC_CACHE[C] = _build_nc(C)
    return _NC_CACHE[C]


def kernel(x, expert_indices, w1, w2, w3):
    global LAST_RESULTS
    from concourse import bass_utils

    x = np.asarray(x, dtype=np.float32)
    idx = np.asarray(expert_indices)
    out_dtype_idx = idx.dtype  # preserved implicitly; output is float32 anyway
    w1 = np.asarray(w1, dtype=np.float32)
    w2 = np.asarray(w2, dtype=np.float32)
    w3 = np.asarray(w3, dtype=np.float32)

    bf16 = ml_dtypes.bfloat16

    # --- host routing: stable-sort the (token, k) slots by expert id ---
    flat = idx.reshape(-1).astype(np.int64)  # slot s = t*TOPK + k -> expert
    order = np.argsort(flat, kind="stable")  # slots grouped by expert
    counts = np.bincount(flat, minlength=E)
    starts = np.zeros(E + 1, dtype=np.int64)
    np.cumsum(counts, out=starts[1:])
    cmax = int(counts.max())
    C = max(512, -(-cmax // 64) * 64)  # pad capacity to a multiple of 64

    nc = _get_nc(C)

    xb = x.astype(bf16)
    in_maps = []
    for e in range(E):
        slots = order[starts[e]:starts[e + 1]]
        tokens = slots // TOPK
        xg = np.zeros((C, DIM), dtype=bf16)
        xg[: len(tokens)] = xb[tokens]
        # [C, DIM] -> [DIM, C] -> [KT, P, C]
        xt = np.ascontiguousarray(xg.T).reshape(KT, P, C)
        w1t = np.ascontiguousarray(w1[e].T.astype(bf16)).reshape(KT, P, INTER)
        w3t = np.ascontiguousarray(w3[e].T.astype(bf16)).reshape(KT, P, INTER)
        w2t = np.ascontiguousarray(w2[e].T.astype(bf16)).reshape(MT, P, DIM)
        in_maps.append({"xt": xt, "w1t": w1t, "w3t": w3t, "w2t": w2t})

    res = bass_utils.run_bass_kernel_spmd(
        nc, in_maps, core_ids=list(range(NCORES)), trace=TRACE
    )
    LAST_RESULTS = res

    out = np.empty((T * TOPK, DIM), dtype=np.float32)
    for e in range(E):
        slots = order[starts[e]:starts[e + 1]]
        yt = res.results[e]["yt"]  # [KT, P, C] f32
        y = yt.reshape(DIM, C)  # y.T
        out[slots] = y[:, : len(slots)].T
    return out.reshape(T, TOPK, DIM)
